# revision 1
# baseline (speedup 1.0000x reference)
"""CrossHeadAttention Trainium2 kernel (8-core SPMD, data+head parallel).

Reference computation (per batch b):
    k = x_enc @ Wk ; v = x_enc @ Wv ; q = x @ Wq        (bias-free linears)
    wei = softmax((q @ k^T) / sqrt(1024))  per head
    out = wei @ v                                        -> [B, T, H, D]

Sharding: 8 cores = 2 batches x 4 head-groups (4 heads each). Each core
receives x[b], x_enc[b] and the 256-column slice of Wq/Wk/Wv for its heads,
and produces out[b][:, :, hg*4:(hg+1)*4, :]. No cross-core communication.

Per-core dataflow (matmuls in float32r = full-rate ~fp32):
  x_enc --PE transpose--> xeT[c,s] --W-stationary matmul--> kT[d,s], vT[d,s]
  x     --PE transpose--> xT[c,t]  -----------------------> qT[d,t]
  vT --PE transpose--> v[s,d] (+ones column for softmax sums)
  S^T[s,t] = k q^T   (K=64 contraction, 2 heads row-packed via tile_position)
  P^T = exp(S^T / 32) on ScalarE (scores are ~N(0,1): no max-subtraction)
  outT[d_aug,t] = v_aug.T @ P^T  (psum-accumulated over s; row 64 = sums)
  out[t,d] = PE-transpose(outT) * 1/sums  (DVE), DMA to HBM.

The transposed activations are built in 512-column chunks that feed their
projections immediately and die, so SBUF holds one rotating 16 KiB/partition
chunk pool instead of 64 KiB static buffers. The kernel runs as two phases
with scoped PSUM pools: a projection phase (6-bank rotating psum; psum->sbuf
rounding copies split between DVE and the otherwise-idle ScalarE) and an
attention phase (4 banks score double-buffer + 2 PV accumulators + 2
finalize banks), with the exp activation table preloaded at t=0.
"""

from contextlib import ExitStack

import numpy as np

import concourse.bacc as bacc
import concourse.tile as tile
from concourse import mybir
from concourse.bass_utils import run_bass_kernel_spmd
from concourse.masks import make_identity

# Problem constants (hardcoded per spec)
B = 2
T = 2048          # query length
S = 2048          # key/value length
C = 1024          # n_embd
H = 16            # total heads
D = 64            # head size
N_CORES = 8
HG = H // (N_CORES // B)       # heads per core = 4
DCORE = HG * D                 # 256 projected dims per core
P = 128                        # partitions
CT = C // P                    # 8 contraction tiles
NPAIR = HG // 2                # 2 head pairs per core
TCH = 512                      # t-chunk width in attention
NTCH = T // TCH                # 4
ST = S // P                    # 16 s-tiles

F32 = mybir.dt.float32
F32R = mybir.dt.float32r
AF = mybir.ActivationFunctionType

SCALE = float(C) ** -0.5       # 1/32, folded into the exp activation


def _build_chain(nc, rows, aux, xtp, src_dram, projs, identity, rowtag):
    """Stream src[t, c] through PE-transpose into rotating [c, 512] chunks,
    and run every projection in `projs` on each chunk as soon as it lands.

    projs: list of (w_slice [P, CT, P] f32r, out_slice_fn(chunk_idx) -> AP).
    """
    for sch in range(src_dram.shape[0] // 512):
        _build_chain_chunk(nc, rows, aux, xtp, src_dram, projs, identity,
                           rowtag, sch, act_copies=True)


def _chain_chunk_pieces(nc, rows, aux, xtp, src_dram, projs, identity,
                        rowtag, sch, act_copies=False):
    """Emission pieces for one 512-wide x^T chunk + its projections.

    Returns a list of zero-arg callables; calling them in order (possibly
    interleaved with other emission) builds the chunk. When act_copies is
    set, half the psum->sbuf copies go to ScalarE instead of DVE (used
    pre-attention while ScalarE is otherwise idle).
    """
    state = {}

    def row_piece(r4):
        def go():
            if r4 == 0:
                state["xc"] = xtp.tile([P, CT, 512], F32R, tag="xch",
                                       name="xch")
            r = sch * 4 + r4
            row = rows.tile([P, C], F32, tag=rowtag, name="row")
            nc.sync.dma_start(out=row, in_=src_dram[r * P:(r + 1) * P, :])
            for cq in range(CT // 4):
                tp = aux.tile([P, 4 * P], F32, tag="aux", name="tp")
                for j in range(4):
                    ct = 4 * cq + j
                    nc.tensor.transpose(
                        tp[:, j * P:(j + 1) * P],
                        row[:, ct * P:(ct + 1) * P], identity)
                if act_copies and cq % 2:
                    copy_fn = nc.scalar.copy
                else:
                    copy_fn = lambda out, in_: nc.vector.tensor_copy(
                        out=out, in_=in_)
                copy_fn(
                    out=state["xc"][:, 4 * cq:4 * cq + 4,
                                    r4 * P:(r4 + 1) * P],
                    in_=tp.rearrange("p (j t) -> p j t", j=4))
        return go

    def proj_piece(w_slice, out_fn):
        def go():
            ps = aux.tile([P, 512], F32, tag="aux", name="ps")
            for ct in range(CT):
                nc.tensor.matmul(
                    ps, w_slice[:, ct, :], state["xc"][:, ct, :],
                    start=(ct == 0), stop=(ct == CT - 1))
            nc.vector.tensor_copy(out=out_fn(sch), in_=ps)
        return go

    return [row_piece(r4) for r4 in range(4)] +            [proj_piece(w, f) for w, f in projs]


def _build_chain_chunk(nc, rows, aux, xtp, src_dram, projs, identity,
                       rowtag, sch, act_copies=False):
    for piece in _chain_chunk_pieces(nc, rows, aux, xtp, src_dram, projs,
                                     identity, rowtag, sch, act_copies):
        piece()


def _build_v_transpose(nc, aux, vT, v_sb, identity, pt):
    """v_sb[s, 2pt:2pt+2, d] = (vT pair tile)^T via PE transpose."""
    for sq in range(ST // 4):
        tp = aux.tile([P, 4 * P], F32, tag="aux", name="tpv")
        for j in range(4):
            st = 4 * sq + j
            nc.tensor.transpose(
                tp[:, j * P:(j + 1) * P],
                vT.bitcast(F32)[:, st * P:(st + 1) * P], identity)
        for j in range(4):
            st = 4 * sq + j
            nc.vector.tensor_copy(
                out=v_sb[:, st, 2 * pt:2 * pt + 2, 0:D],
                in_=tp[:, j * P:(j + 1) * P].rearrange(
                    "p (h d) -> p h d", h=2))


def _build_attention_tch(nc, spsum, pvpools, aux, psb, otp, fin,
                         kT, qT, v_sb, identity, out, pair, tch,
                         interleave=()):
    """Attention st-loop for one head pair and one t-chunk -> oT tiles.

    `interleave`: emission pieces (e.g. next chunk's build) spliced between
    st iterations so the static schedule overlaps them with the exp stream.
    """
    if True:
        interleave = list(interleave)
        tsl = slice(tch * TCH, (tch + 1) * TCH)
        pv_ps = [pvpools[h2].tile([D + 1, TCH], F32, tag=f"pv{h2}",
                                  name=f"pv{h2}")
                 for h2 in range(2)]
        for st in range(ST):
            s_ps = spsum.tile([P, 2 * TCH], F32, tag="s", name="s_ps")
            for h2 in range(2):
                nc.tensor.matmul(
                    s_ps[:, h2 * TCH:(h2 + 1) * TCH],
                    kT[h2 * D:(h2 + 1) * D, pair, st * P:(st + 1) * P],
                    qT[h2 * D:(h2 + 1) * D, pair, tsl],
                    start=True, stop=True,
                    tile_position=(h2 * D, 0),
                )
            p_sb = psb.tile([P, 2 * TCH], F32R, tag="p", name="p_sb")
            nc.scalar.activation(out=p_sb, in_=s_ps, func=AF.Exp, scale=SCALE)
            for h2 in range(2):
                nc.tensor.matmul(
                    pv_ps[h2],
                    v_sb[:, st, 2 * pair + h2, :],
                    p_sb[:, h2 * TCH:(h2 + 1) * TCH],
                    start=(st == 0), stop=(st == ST - 1),
                )
            if interleave and st % 2 == 1:
                interleave.pop(0)()
        for piece in interleave:
            piece()
        oT = []
        for h2 in range(2):
            t_ = otp.tile([D + 1, TCH], F32, tag=f"oT{pair}{h2}",
                          name=f"oT{pair}{h2}")
            nc.vector.tensor_copy(out=t_, in_=pv_ps[h2])
            oT.append(t_)
        return oT


def _build_finalize_tch(nc, spsum, fin, oT, identity, out, pair, tch):
    """Transpose oT heads into a spsum bank, normalize by sums, store.

    Uses the spsum pool (not aux) so the next chunk-build's transposes are
    never serialized behind this tail work.
    """
    for sub in range(TCH // P):
        tt = tch * (TCH // P) + sub
        o_tile = fin.tile([P, 2 * D], F32, tag="o", name="o_tile")
        tp = spsum.tile([P, 2 * (D + 1)], F32, tag="ft", name="ft")
        for h2 in range(2):
            nc.tensor.transpose(
                tp[:, h2 * (D + 1):(h2 + 1) * (D + 1)],
                oT[h2][:, sub * P:(sub + 1) * P],
                identity[0:D + 1, 0:D + 1])
        tph = tp.rearrange("p (h e) -> p h e", h=2)
        r2 = fin.tile([P, 2], F32, tag="r", name="r2")
        nc.vector.reciprocal(out=r2, in_=tph[:, :, D])
        for h2 in range(2):
            nc.vector.tensor_scalar_mul(
                out=o_tile[:, h2 * D:(h2 + 1) * D],
                in0=tph[:, h2, 0:D], scalar1=r2[:, h2:h2 + 1])
        # SWDGE: keeps this dependent store out of SP's in-order
        # stream so it cannot head-of-line-block later row loads
        nc.gpsimd.dma_start(
            out=out[tt * P:(tt + 1) * P,
                    pair * 2 * D:(pair + 1) * 2 * D],
            in_=o_tile)


def _attention_phase(nc, tc, kT, qT, v_sb, identity, out,
                     psb, otp, fin):
    with tc.tile_pool(name="spsum", bufs=2, space="PSUM") as spsum, \
         tc.tile_pool(name="pvpsum0", bufs=1, space="PSUM") as pvp0, \
         tc.tile_pool(name="pvpsum1", bufs=1, space="PSUM") as pvp1, \
         tc.tile_pool(name="ftpsum", bufs=2, space="PSUM") as ftp:
        pvpools = (pvp0, pvp1)
        for tch in range(NTCH):
            oT0 = _build_attention_tch(
                nc, spsum, pvpools, None, psb, otp, fin,
                kT, qT, v_sb, identity, out, 0, tch)
            # pair-0 finalize emitted before pair-1 attention so its
            # transposes/stores run under pair-1's exp stream
            _build_finalize_tch(nc, ftp, fin, oT0, identity, out, 0, tch)
            oT1 = _build_attention_tch(
                nc, spsum, pvpools, None, psb, otp, fin,
                kT, qT, v_sb, identity, out, 1, tch)
            _build_finalize_tch(nc, ftp, fin, oT1, identity, out, 1, tch)


def _build_body(nc, tc, x, xe, wq, wk, wv, out):
    with ExitStack() as ctx:
        consts = ctx.enter_context(tc.tile_pool(name="consts", bufs=1))
        big = ctx.enter_context(tc.tile_pool(name="big", bufs=1))
        psb = ctx.enter_context(tc.tile_pool(name="psb", bufs=3))
        otp = ctx.enter_context(tc.tile_pool(name="otp", bufs=2))
        fin = ctx.enter_context(tc.tile_pool(name="fin", bufs=3))

        identity = consts.tile([P, P], F32)
        make_identity(nc, identity)
        # prime the ScalarE exp table at t=0 so the ~2.7us ACT_TABLE_LOAD is
        # off the critical path of the first real exp
        dummy = consts.tile([1, 2], F32)
        nc.vector.memset(dummy, 0.0)
        nc.scalar.activation(out=dummy, in_=dummy, func=AF.Exp)

        kT = big.tile([P, NPAIR, S], F32R, tag="kT")
        qT = big.tile([P, NPAIR, T], F32R, tag="qT")
        vT0 = big.tile([P, S], F32R, tag="vT0")
        vT1 = big.tile([P, S], F32R, tag="vT1")
        # v, with a ones column appended per head (col D) for softmax sums
        v_sb = big.tile([P, ST, HG, D + 1], F32R, tag="v_sb")
        nc.vector.memset(v_sb[:, :, :, D].bitcast(F32), 1.0)

        with tc.tile_pool(name="xtp", bufs=2) as xtp, \
             tc.tile_pool(name="rows", bufs=3) as rows, \
             tc.tile_pool(name="wpool", bufs=1) as wpool:

            # weights: DMA f32 staging -> DVE rounding copy -> f32r
            w_sbs = {}
            for name, wdram in (("wk", wk), ("wv", wv), ("wq", wq)):
                stage = wpool.tile([P, CT, DCORE], F32, tag="wstage",
                                   name="wstage")
                nc.gpsimd.dma_start(
                    out=stage, in_=wdram.rearrange("(ct p) d -> p ct d", p=P))
                wsb = wpool.tile([P, CT, DCORE], F32R, tag=f"{name}_sb",
                                 name=f"{name}_sb")
                nc.vector.tensor_copy(out=wsb, in_=stage)
                w_sbs[name] = wsb

            def _dsl(wname, dt_):
                return w_sbs[wname][:, :, dt_ * P:(dt_ + 1) * P]

            with tc.tile_pool(name="chainps", bufs=6, space="PSUM") as aux:
                # xe chain: k^T and v^T for both pairs, chunk-streamed
                _build_chain(
                    nc, rows, aux, xtp, xe,
                    [(_dsl("wk", 0),
                      lambda s: kT[:, 0, s * 512:(s + 1) * 512]),
                     (_dsl("wv", 0),
                      lambda s: vT0[:, s * 512:(s + 1) * 512]),
                     (_dsl("wk", 1),
                      lambda s: kT[:, 1, s * 512:(s + 1) * 512]),
                     (_dsl("wv", 1),
                      lambda s: vT1[:, s * 512:(s + 1) * 512])],
                    identity, "row")
                _build_v_transpose(nc, aux, vT0, v_sb, identity, 0)
                _build_v_transpose(nc, aux, vT1, v_sb, identity, 1)

                # x chain: q^T for both pairs
                qproj = [(_dsl("wq", 0),
                          lambda s: qT[:, 0, s * 512:(s + 1) * 512]),
                         (_dsl("wq", 1),
                          lambda s: qT[:, 1, s * 512:(s + 1) * 512])]
                _build_chain(nc, rows, aux, xtp, x, qproj, identity, "row")

            _attention_phase(nc, tc, kT, qT, v_sb, identity, out,
                             psb, otp, fin)


def build_program():
    nc = bacc.Bacc("TRN2", target_bir_lowering=False, debug=False,
                   num_devices=N_CORES)

    x = nc.dram_tensor("x", [T, C], F32, kind="ExternalInput").ap()
    xe = nc.dram_tensor("xe", [S, C], F32, kind="ExternalInput").ap()
    wq = nc.dram_tensor("wq", [C, DCORE], F32, kind="ExternalInput").ap()
    wk = nc.dram_tensor("wk", [C, DCORE], F32, kind="ExternalInput").ap()
    wv = nc.dram_tensor("wv", [C, DCORE], F32, kind="ExternalInput").ap()
    out = nc.dram_tensor("out", [T, DCORE], F32, kind="ExternalOutput").ap()

    with tile.TileContext(nc) as tc:
        _build_body(nc, tc, x, xe, wq, wk, wv, out)
    nc.compile()
    return nc


_NC_CACHE = None


def _get_program():
    global _NC_CACHE
    if _NC_CACHE is None:
        _NC_CACHE = build_program()
    return _NC_CACHE


def kernel(x_enc, x, Wk, Wq, Wv):
    x_enc = np.asarray(x_enc, dtype=np.float32)
    x = np.asarray(x, dtype=np.float32)
    Wk = np.asarray(Wk, dtype=np.float32)
    Wq = np.asarray(Wq, dtype=np.float32)
    Wv = np.asarray(Wv, dtype=np.float32)

    nc = _get_program()
    in_maps = []
    for core in range(N_CORES):
        b, hg = divmod(core, N_CORES // B)
        csl = slice(hg * DCORE, (hg + 1) * DCORE)
        in_maps.append({
            "x": np.ascontiguousarray(x[b]),
            "xe": np.ascontiguousarray(x_enc[b]),
            "wq": np.ascontiguousarray(Wq[:, csl]),
            "wk": np.ascontiguousarray(Wk[:, csl]),
            "wv": np.ascontiguousarray(Wv[:, csl]),
        })
    res = run_bass_kernel_spmd(nc, in_maps, list(range(N_CORES)))

    full = np.empty((B, T, H, D), dtype=np.float32)
    for core in range(N_CORES):
        b, hg = divmod(core, N_CORES // B)
        o = res.results[core]["out"].reshape(T, HG, D)
        full[b, :, hg * HG:(hg + 1) * HG, :] = o
    return full



# revision 6
# speedup vs baseline: 1.2172x; 1.2172x over previous
"""CrossHeadAttention Trainium2 kernel (8-core SPMD, data+head parallel).

Reference computation (per batch b):
    k = x_enc @ Wk ; v = x_enc @ Wv ; q = x @ Wq        (bias-free linears)
    wei = softmax((q @ k^T) / sqrt(1024))  per head
    out = wei @ v                                        -> [B, T, H, D]

Sharding: 8 cores = 2 batches x 4 head-groups (4 heads each). Each core
receives x[b], x_enc[b] (host-cast to bf16) and the 256-column slice of
Wq/Wk/Wv for its heads, and produces the unnormalized attention numerator
plus the softmax denominator; the host divides and gathers.

Per-core dataflow (all matmuls bf16 = 1 col/cycle on the PE):
  x, x_enc --HW DMA-transpose (bf16, 16x128 xbar tiles)--> xT/xeT in SBUF
    (zero PE cost; the PE never transposes activations)
  qT/kT[d,t] = W-stationary matmuls; psum->sbuf bf16 rounding on DVE
  vT chunks --DMA-transpose--> v_sb[s, st, head, d] (+ones column at d=64
    so the PV matmul also produces the softmax denominator)
  S^T[s,t] = kT^T qT per head (K=64, two heads row-packed in partitions)
  p = exp(S/32): split 10/16 exact on ScalarE (act table, psum->sbuf bf16)
    and 6/16 on DVE via a calibrated Schraudolph bit-trick: bf16 bits of
    exp2(x) ~ int16(round(128*log2(e)*x/32 + 16250)); the constant-factor
    part of its bias cancels in the softmax ratio, leaving ~1% noise on
    3/8 of the weights (measured end-to-end err ~7e-3 vs the 2e-2 gate)
  num^T[t, d+1] = p-stationary PV matmuls: out[t-block, 65] accumulated
    over s-tiles into a single psum bank (DVE pre-memset + start=False
    so four 65-wide accumulators share one bank without zero-region
    clobber); col 64 = denominator. DVE copies psum->sbuf, SWDGE stores.

Engine budget per core (TimelineSim): PE ~296k cycles (123us) = proj 98k
+ scores 131k + PV 67k; ACT ~85us exp; DVE ~85us (schrau + copies); DMA
~45us. The kernel is PE-bound; exp hides under the matmul stream.
"""

from contextlib import ExitStack

import numpy as np
import ml_dtypes

import concourse.bacc as bacc
import concourse.tile as tile
from concourse import mybir
from concourse.bass_utils import run_bass_kernel_spmd

# Problem constants (hardcoded per spec)
B = 2
T = 2048          # query length
S = 2048          # key/value length
C = 1024          # n_embd
H = 16            # total heads
D = 64            # head size
N_CORES = 8
HG = H // (N_CORES // B)       # heads per core = 4
DCORE = HG * D                 # 256 projected dims per core
P = 128                        # partitions
CT = C // P                    # 8 contraction tiles
NPAIR = HG // 2                # 2 head pairs per core
TCH = 512                      # t-chunk width in attention
NTCH = T // TCH                # 4
ST = S // P                    # 16 s-tiles
NCH = 4                        # 512-row input chunks

F32 = mybir.dt.float32
BF16 = mybir.dt.bfloat16
I16 = mybir.dt.int16
AF = mybir.ActivationFunctionType

SCALE = float(C) ** -0.5       # 1/32, folded into the exp activation

# Schraudolph exp for the DVE share: bf16 bitpattern of exp(s*SCALE) ~
# round(A*s + B); B = 16256 (bf16 exponent bias<<7) + m, m=-6 calibrated
# end-to-end for round-to-nearest int16 conversion.
SCH_A = 128.0 * float(np.log2(np.e)) * SCALE
SCH_B = 16256.0 - 6.0
N_ACT_SP = 5                   # s-tile-pairs 0..4 on ScalarE (exact exp)


def _build_body(nc, tc, x, xe, wq, wk, wv, o):
    with ExitStack() as ctx:
        big = ctx.enter_context(tc.tile_pool(name="big", bufs=1))

        # per-chunk contiguous tiles: the DMA xbar transpose requires a
        # contiguous output access pattern
        xT = [big.tile([P, CT, TCH], BF16, tag=f"xT{c}", name=f"xT{c}")
              for c in range(NCH)]
        xeT = [big.tile([P, CT, TCH], BF16, tag=f"xeT{c}", name=f"xeT{c}")
               for c in range(NCH)]
        kT = big.tile([P, NPAIR, S], BF16, tag="kT")
        qT = big.tile([P, NPAIR, T], BF16, tag="qT")
        # v, with a ones column appended per head (col D) for softmax sums
        v_sb = big.tile([P, ST, HG, D + 1], BF16, tag="v_sb")
        nc.vector.memset(v_sb[:, :, :, D], 1.0)

        # prime the ScalarE exp table at t=0 so the table load is off the
        # critical path of the first real exp
        dummy = big.tile([1, 2], F32, tag="dummy")
        nc.vector.memset(dummy, 0.0)
        nc.scalar.activation(out=dummy, in_=dummy, func=AF.Exp)

        # weights, host-staged as [P, CT, DCORE] bf16: direct DMA
        w_sbs = {}
        for name, wdram in (("wq", wq), ("wk", wk), ("wv", wv)):
            wsb = big.tile([P, CT, DCORE], BF16, tag=f"{name}_sb")
            nc.sync.dma_start(out=wsb, in_=wdram)
            w_sbs[name] = wsb

        # activation transposes DRAM->SBUF on the DMA xbar, chunked for
        # pipelining with the projections
        for c in range(NCH):
            nc.sync.dma_start_transpose(xT[c], x[c * TCH:(c + 1) * TCH, :])
        for c in range(NCH):
            nc.sync.dma_start_transpose(xeT[c], xe[c * TCH:(c + 1) * TCH, :])

        def proj(ps, wname, pair, src, c):
            w = w_sbs[wname]
            for ct in range(CT):
                nc.tensor.matmul(
                    ps, w[:, ct, pair * P:(pair + 1) * P], src[c][:, ct, :],
                    start=(ct == 0), stop=(ct == CT - 1))

        with tc.tile_pool(name="pps", bufs=3, space="PSUM") as pps, \
             tc.tile_pool(name="vtc", bufs=2) as vtc:
            for c in range(NCH):
                csl = slice(c * TCH, (c + 1) * TCH)
                for pair in range(NPAIR):
                    ps = pps.tile([P, TCH], F32, tag="pps", name="qps")
                    proj(ps, "wq", pair, xT, c)
                    nc.vector.tensor_copy(out=qT[:, pair, csl], in_=ps)
            for c in range(NCH):
                csl = slice(c * TCH, (c + 1) * TCH)
                for pair in range(NPAIR):
                    ps = pps.tile([P, TCH], F32, tag="pps", name="kps")
                    proj(ps, "wk", pair, xeT, c)
                    nc.vector.tensor_copy(out=kT[:, pair, csl], in_=ps)
                for pair in range(NPAIR):
                    ps = pps.tile([P, TCH], F32, tag="pps", name="vps")
                    proj(ps, "wv", pair, xeT, c)
                    vt = vtc.tile([P, TCH], BF16, tag="vt", name="vt")
                    nc.vector.tensor_copy(out=vt, in_=ps)
                    for h2 in range(2):
                        # [d, s-chunk] -> contiguous [s%128, st, d], then a
                        # DVE copy into v_sb's 65-strided head slot
                        vtr = vtc.tile([P, 4, D], BF16, tag="vtr",
                                       name="vtr")
                        nc.sync.dma_start_transpose(
                            vtr, vt[h2 * D:(h2 + 1) * D, :])
                        nc.vector.tensor_copy(
                            out=v_sb[:, c * 4:(c + 1) * 4, 2 * pair + h2,
                                     0:D],
                            in_=vtr)

        with tc.tile_pool(name="sps", bufs=2, space="PSUM") as sps, \
             tc.tile_pool(name="pvps", bufs=2, space="PSUM") as pvps, \
             tc.tile_pool(name="psb", bufs=3) as psb, \
             tc.tile_pool(name="osb", bufs=3) as osb:
            for tch in range(NTCH):
                tsl = slice(tch * TCH, (tch + 1) * TCH)
                for pair in range(NPAIR):
                    for h2 in range(2):
                        h = 2 * pair + h2
                        # four 65-wide accumulators packed in one psum bank:
                        # memset + start=False avoids per-chain zero-region
                        # clobber of neighbours
                        pv = pvps.tile([P, 4, D + 1], F32, tag="pv",
                                       name="pv")
                        nc.vector.memset(pv, 0.0)
                        for sp in range(ST // 2):
                            s_ps = sps.tile([P, 2, TCH], F32, tag="s",
                                            name="s_ps")
                            for j in range(2):
                                st = 2 * sp + j
                                nc.tensor.matmul(
                                    s_ps[:, j, :],
                                    kT[h2 * D:(h2 + 1) * D, pair,
                                       st * P:(st + 1) * P],
                                    qT[h2 * D:(h2 + 1) * D, pair, tsl],
                                    start=True, stop=True,
                                    tile_position=(h2 * D, 0))
                            p_t = psb.tile([P, 2, TCH], BF16, tag="p",
                                           name="p_t")
                            if sp < N_ACT_SP:
                                nc.scalar.activation(
                                    out=p_t, in_=s_ps, func=AF.Exp,
                                    scale=SCALE)
                            else:
                                nc.vector.tensor_scalar(
                                    out=p_t.bitcast(I16), in0=s_ps,
                                    scalar1=SCH_A, scalar2=SCH_B,
                                    op0=mybir.AluOpType.mult,
                                    op1=mybir.AluOpType.add)
                            for j in range(2):
                                st = 2 * sp + j
                                for tb in range(4):
                                    nc.tensor.matmul(
                                        pv[:, tb, :],
                                        p_t[:, j, tb * P:(tb + 1) * P],
                                        v_sb[:, st, h, :],
                                        start=False, stop=(st == ST - 1),
                                        skip_group_check=True)
                        o_t = osb.tile([P, 4, D + 1], F32, tag="o",
                                       name="o_t")
                        nc.vector.tensor_copy(out=o_t, in_=pv)
                        # SWDGE keeps stores off the SP queue feeding loads
                        nc.gpsimd.dma_start(out=o[tch, h], in_=o_t)


def build_program():
    nc = bacc.Bacc("TRN2", target_bir_lowering=False, debug=False,
                   num_devices=N_CORES)

    x = nc.dram_tensor("x", [T, C], BF16, kind="ExternalInput").ap()
    xe = nc.dram_tensor("xe", [S, C], BF16, kind="ExternalInput").ap()
    wq = nc.dram_tensor("wq", [P, CT, DCORE], BF16, kind="ExternalInput").ap()
    wk = nc.dram_tensor("wk", [P, CT, DCORE], BF16, kind="ExternalInput").ap()
    wv = nc.dram_tensor("wv", [P, CT, DCORE], BF16, kind="ExternalInput").ap()
    # per (tch, h): [t%512 partition, t-block, d+1]; col D = softmax denom
    o = nc.dram_tensor("o", [NTCH, HG, P, 4, D + 1], F32,
                       kind="ExternalOutput").ap()

    with tile.TileContext(nc) as tc:
        _build_body(nc, tc, x, xe, wq, wk, wv, o)
    nc.compile()
    return nc


_NC_CACHE = None


def _get_program():
    global _NC_CACHE
    if _NC_CACHE is None:
        _NC_CACHE = build_program()
    return _NC_CACHE


def _stage_w(wfull, csl):
    # [1024, 256] slice -> [P, CT, DCORE] bf16 with w[p, ct, d] = W[ct*128+p, d]
    wslc = np.ascontiguousarray(wfull[:, csl]).astype(ml_dtypes.bfloat16)
    return np.ascontiguousarray(wslc.reshape(CT, P, DCORE).transpose(1, 0, 2))


def kernel(x_enc, x, Wk, Wq, Wv):
    x_enc = np.asarray(x_enc, dtype=np.float32)
    x = np.asarray(x, dtype=np.float32)
    Wk = np.asarray(Wk, dtype=np.float32)
    Wq = np.asarray(Wq, dtype=np.float32)
    Wv = np.asarray(Wv, dtype=np.float32)

    nc = _get_program()
    in_maps = []
    for core in range(N_CORES):
        b, hg = divmod(core, N_CORES // B)
        csl = slice(hg * DCORE, (hg + 1) * DCORE)
        in_maps.append({
            "x": np.ascontiguousarray(x[b]).astype(ml_dtypes.bfloat16),
            "xe": np.ascontiguousarray(x_enc[b]).astype(ml_dtypes.bfloat16),
            "wq": _stage_w(Wq, csl),
            "wk": _stage_w(Wk, csl),
            "wv": _stage_w(Wv, csl),
        })
    res = run_bass_kernel_spmd(nc, in_maps, list(range(N_CORES)))

    full = np.empty((B, T, H, D), dtype=np.float32)
    for core in range(N_CORES):
        b, hg = divmod(core, N_CORES // B)
        o = res.results[core]["o"]          # [NTCH, HG, P, 4, D+1] f32
        num = o[..., :D]
        den = o[..., D]
        out = num / den[..., None]          # [tch, h, p, tb, d]
        out = out.transpose(0, 3, 2, 1, 4).reshape(T, HG, D)
        full[b, :, hg * HG:(hg + 1) * HG, :] = out
    return full


# revision 8
# speedup vs baseline: 1.4083x; 1.1570x over previous
"""CrossHeadAttention Trainium2 kernel (8-core SPMD, data+head parallel).

Reference computation (per batch b):
    k = x_enc @ Wk ; v = x_enc @ Wv ; q = x @ Wq        (bias-free linears)
    wei = softmax((q @ k^T) / sqrt(1024))  per head
    out = wei @ v                                        -> [B, T, H, D]

Sharding: 8 cores = 2 batches x 4 head-groups (4 heads each). Each core
receives x[b], x_enc[b] (host-cast to bf16) and the 256-column slice of
Wq/Wk/Wv for its heads, and produces the unnormalized attention numerator
plus the softmax denominator; the host divides and gathers.

Per-core dataflow (all matmuls bf16 = 1 col/cycle on the PE):
  x, x_enc --HW DMA-transpose (bf16, 16x128 xbar tiles)--> xT/xeT in SBUF
    (zero PE cost; the PE never transposes activations)
  qT/kT[d,t] = W-stationary matmuls; psum->sbuf bf16 rounding on DVE
  vT chunks --DMA-transpose--> v_sb[s, st, head, d] (+ones column at d=64
    so the PV matmul also produces the softmax denominator)
  S^T[s,t] = kT^T qT per head (K=64, two heads row-packed in partitions)
  p = exp(S/32): split 10/16 exact on ScalarE (act table, psum->sbuf bf16)
    and 6/16 on DVE via a calibrated Schraudolph bit-trick: bf16 bits of
    exp2(x) ~ int16(round(128*log2(e)*x/32 + 16250)); the constant-factor
    part of its bias cancels in the softmax ratio, leaving ~1% noise on
    3/8 of the weights (measured end-to-end err ~7e-3 vs the 2e-2 gate)
  num^T[t, d+1] = p-stationary PV matmuls: out[t-block, 65] accumulated
    over s-tiles into a single psum bank (DVE pre-memset + start=False
    so four 65-wide accumulators share one bank without zero-region
    clobber); col 64 = denominator. DVE copies psum->sbuf, SWDGE stores.

Engine budget per core (TimelineSim): PE ~296k cycles (123us) = proj 98k
+ scores 131k + PV 67k; ACT ~85us exp; DVE ~85us (schrau + copies); DMA
~45us. The kernel is PE-bound; exp hides under the matmul stream.
"""

from contextlib import ExitStack

import numpy as np
import ml_dtypes

import concourse.bacc as bacc
import concourse.tile as tile
from concourse import mybir
from concourse.bass_utils import run_bass_kernel_spmd

# Problem constants (hardcoded per spec)
B = 2
T = 2048          # query length
S = 2048          # key/value length
C = 1024          # n_embd
H = 16            # total heads
D = 64            # head size
N_CORES = 8
HG = H // (N_CORES // B)       # heads per core = 4
DCORE = HG * D                 # 256 projected dims per core
P = 128                        # partitions
CT = C // P                    # 8 contraction tiles
NPAIR = HG // 2                # 2 head pairs per core
TCH = 512                      # t-chunk width in attention
NTCH = T // TCH                # 4
ST = S // P                    # 16 s-tiles
NCH = 4                        # 512-row input chunks

F32 = mybir.dt.float32
BF16 = mybir.dt.bfloat16
I16 = mybir.dt.int16
AF = mybir.ActivationFunctionType

SCALE = float(C) ** -0.5       # 1/32, folded into the exp activation

# Schraudolph exp for the DVE share: bf16 bitpattern of exp(s*SCALE) ~
# round(A*s + B); B = 16256 (bf16 exponent bias<<7) + m, m=-6 calibrated
# end-to-end for round-to-nearest int16 conversion.
SCH_A = 128.0 * float(np.log2(np.e)) * SCALE
SCH_B = 16256.0 - 6.0
N_ACT_SP = 5                   # s-tile-pairs 0..4 on ScalarE (exact exp)


def _build_body(nc, tc, x, xe, wq, wk, wv, o):
    with ExitStack() as ctx:
        big = ctx.enter_context(tc.tile_pool(name="big", bufs=1))

        # per-chunk contiguous tiles: the DMA xbar transpose requires a
        # contiguous output access pattern
        xT = [big.tile([P, CT, TCH], BF16, tag=f"xT{c}", name=f"xT{c}")
              for c in range(NCH)]
        xeT = [big.tile([P, CT, TCH], BF16, tag=f"xeT{c}", name=f"xeT{c}")
               for c in range(NCH)]
        kT = big.tile([P, NPAIR, S], BF16, tag="kT")
        qT = big.tile([P, NPAIR, T], BF16, tag="qT")
        # v, with a ones column appended per head (col D) for softmax sums
        v_sb = big.tile([P, ST, HG, D + 1], BF16, tag="v_sb")
        nc.vector.memset(v_sb[:, :, :, D], 1.0)

        # prime the ScalarE exp table at t=0 so the table load is off the
        # critical path of the first real exp
        dummy = big.tile([1, 2], F32, tag="dummy")
        nc.vector.memset(dummy, 0.0)
        nc.scalar.activation(out=dummy, in_=dummy, func=AF.Exp)

        # weights, host-staged as [P, CT, DCORE] bf16: direct DMA
        w_sbs = {}
        for name, wdram in (("wq", wq), ("wk", wk), ("wv", wv)):
            wsb = big.tile([P, CT, DCORE], BF16, tag=f"{name}_sb")
            nc.sync.dma_start(out=wsb, in_=wdram)
            w_sbs[name] = wsb

        # activation transposes DRAM->SBUF on the DMA xbar, chunked for
        # pipelining with the projections
        for c in range(NCH):
            nc.sync.dma_start_transpose(xT[c], x[c * TCH:(c + 1) * TCH, :])
        for c in range(NCH):
            nc.sync.dma_start_transpose(xeT[c], xe[c * TCH:(c + 1) * TCH, :])

        def proj(ps, wname, pair, src, c):
            w = w_sbs[wname]
            for ct in range(CT):
                nc.tensor.matmul(
                    ps, w[:, ct, pair * P:(pair + 1) * P], src[c][:, ct, :],
                    start=(ct == 0), stop=(ct == CT - 1))

        with tc.tile_pool(name="pps", bufs=3, space="PSUM") as pps, \
             tc.tile_pool(name="vtc", bufs=2) as vtc:
            for c in range(NCH):
                csl = slice(c * TCH, (c + 1) * TCH)
                for pair in range(NPAIR):
                    ps = pps.tile([P, TCH], F32, tag="pps", name="qps")
                    proj(ps, "wq", pair, xT, c)
                    nc.vector.tensor_copy(out=qT[:, pair, csl], in_=ps)
            for c in range(NCH):
                csl = slice(c * TCH, (c + 1) * TCH)
                for pair in range(NPAIR):
                    ps = pps.tile([P, TCH], F32, tag="pps", name="kps")
                    proj(ps, "wk", pair, xeT, c)
                    nc.vector.tensor_copy(out=kT[:, pair, csl], in_=ps)
                for pair in range(NPAIR):
                    ps = pps.tile([P, TCH], F32, tag="pps", name="vps")
                    proj(ps, "wv", pair, xeT, c)
                    vt = vtc.tile([P, TCH], BF16, tag="vt", name="vt")
                    nc.vector.tensor_copy(out=vt, in_=ps)
                    for h2 in range(2):
                        # [d, s-chunk] -> contiguous [s%128, st, d], then a
                        # DVE copy into v_sb's 65-strided head slot
                        vtr = vtc.tile([P, 4, D], BF16, tag="vtr",
                                       name="vtr")
                        nc.sync.dma_start_transpose(
                            vtr, vt[h2 * D:(h2 + 1) * D, :])
                        nc.vector.tensor_copy(
                            out=v_sb[:, c * 4:(c + 1) * 4, 2 * pair + h2,
                                     0:D],
                            in_=vtr)

        # Attention: flat software-pipelined loop over (group, s-tile-pair)
        # units. Scores are emitted LOOK=2 units ahead of the exp/PV that
        # consume them, so the PE never sits behind the ~1us exp latency;
        # the PE stream is ... sc(i+2) pv(i) sc(i+3) pv(i+1) ...
        LOOK = 2
        NSP = ST // 2
        units = [(tch, pair, h2, sp)
                 for tch in range(NTCH)
                 for pair in range(NPAIR)
                 for h2 in range(2)
                 for sp in range(NSP)]

        with tc.tile_pool(name="sps", bufs=LOOK + 1, space="PSUM") as sps, \
             tc.tile_pool(name="pvps", bufs=2, space="PSUM") as pvps, \
             tc.tile_pool(name="psb", bufs=LOOK + 2) as psb, \
             tc.tile_pool(name="osb", bufs=3) as osb:
            s_tiles = {}
            pv_tile = [None]

            def emit_scores(u):
                tch, pair, h2, sp = u
                tsl = slice(tch * TCH, (tch + 1) * TCH)
                s_ps = sps.tile([P, 2, TCH], F32, tag="s", name="s_ps")
                for j in range(2):
                    st = 2 * sp + j
                    nc.tensor.matmul(
                        s_ps[:, j, :],
                        kT[h2 * D:(h2 + 1) * D, pair, st * P:(st + 1) * P],
                        qT[h2 * D:(h2 + 1) * D, pair, tsl],
                        start=True, stop=True,
                        tile_position=(h2 * D, 0))
                s_tiles[u] = s_ps

            for u in units[:LOOK]:
                emit_scores(u)
            for i, u in enumerate(units):
                tch, pair, h2, sp = u
                h = 2 * pair + h2
                if sp == 0:
                    # four 65-wide accumulators packed in one psum bank:
                    # DVE memset + start=False chains avoid the per-chain
                    # 2KB zero-region clobbering bank neighbours
                    pv_tile[0] = pvps.tile([P, 4, D + 1], F32, tag="pv",
                                           name="pv")
                    nc.vector.memset(pv_tile[0], 0.0)
                if i + LOOK < len(units):
                    emit_scores(units[i + LOOK])
                s_ps = s_tiles.pop(u)
                p_t = psb.tile([P, 2, TCH], BF16, tag="p", name="p_t")
                if sp < N_ACT_SP:
                    nc.scalar.activation(out=p_t, in_=s_ps, func=AF.Exp,
                                         scale=SCALE)
                else:
                    nc.vector.tensor_scalar(
                        out=p_t.bitcast(I16), in0=s_ps,
                        scalar1=SCH_A, scalar2=SCH_B,
                        op0=mybir.AluOpType.mult,
                        op1=mybir.AluOpType.add)
                pv = pv_tile[0]
                for j in range(2):
                    st = 2 * sp + j
                    for tb in range(4):
                        nc.tensor.matmul(
                            pv[:, tb, :],
                            p_t[:, j, tb * P:(tb + 1) * P],
                            v_sb[:, st, h, :],
                            start=False, stop=(st == ST - 1),
                            skip_group_check=True)
                if sp == NSP - 1:
                    o_t = osb.tile([P, 4, D + 1], F32, tag="o", name="o_t")
                    nc.vector.tensor_copy(out=o_t, in_=pv)
                    # SWDGE keeps stores off the SP queue feeding loads
                    nc.gpsimd.dma_start(out=o[tch, h], in_=o_t)


def build_program():
    nc = bacc.Bacc("TRN2", target_bir_lowering=False, debug=False,
                   num_devices=N_CORES)

    x = nc.dram_tensor("x", [T, C], BF16, kind="ExternalInput").ap()
    xe = nc.dram_tensor("xe", [S, C], BF16, kind="ExternalInput").ap()
    wq = nc.dram_tensor("wq", [P, CT, DCORE], BF16, kind="ExternalInput").ap()
    wk = nc.dram_tensor("wk", [P, CT, DCORE], BF16, kind="ExternalInput").ap()
    wv = nc.dram_tensor("wv", [P, CT, DCORE], BF16, kind="ExternalInput").ap()
    # per (tch, h): [t%512 partition, t-block, d+1]; col D = softmax denom
    o = nc.dram_tensor("o", [NTCH, HG, P, 4, D + 1], F32,
                       kind="ExternalOutput").ap()

    with tile.TileContext(nc) as tc:
        _build_body(nc, tc, x, xe, wq, wk, wv, o)
    nc.compile()
    return nc


_NC_CACHE = None


def _get_program():
    global _NC_CACHE
    if _NC_CACHE is None:
        _NC_CACHE = build_program()
    return _NC_CACHE


def _stage_w(wfull, csl):
    # [1024, 256] slice -> [P, CT, DCORE] bf16 with w[p, ct, d] = W[ct*128+p, d]
    wslc = np.ascontiguousarray(wfull[:, csl]).astype(ml_dtypes.bfloat16)
    return np.ascontiguousarray(wslc.reshape(CT, P, DCORE).transpose(1, 0, 2))


def kernel(x_enc, x, Wk, Wq, Wv):
    x_enc = np.asarray(x_enc, dtype=np.float32)
    x = np.asarray(x, dtype=np.float32)
    Wk = np.asarray(Wk, dtype=np.float32)
    Wq = np.asarray(Wq, dtype=np.float32)
    Wv = np.asarray(Wv, dtype=np.float32)

    nc = _get_program()
    in_maps = []
    for core in range(N_CORES):
        b, hg = divmod(core, N_CORES // B)
        csl = slice(hg * DCORE, (hg + 1) * DCORE)
        in_maps.append({
            "x": np.ascontiguousarray(x[b]).astype(ml_dtypes.bfloat16),
            "xe": np.ascontiguousarray(x_enc[b]).astype(ml_dtypes.bfloat16),
            "wq": _stage_w(Wq, csl),
            "wk": _stage_w(Wk, csl),
            "wv": _stage_w(Wv, csl),
        })
    res = run_bass_kernel_spmd(nc, in_maps, list(range(N_CORES)))

    full = np.empty((B, T, H, D), dtype=np.float32)
    for core in range(N_CORES):
        b, hg = divmod(core, N_CORES // B)
        o = res.results[core]["o"]          # [NTCH, HG, P, 4, D+1] f32
        num = o[..., :D]
        den = o[..., D]
        out = num / den[..., None]          # [tch, h, p, tb, d]
        out = out.transpose(0, 3, 2, 1, 4).reshape(T, HG, D)
        full[b, :, hg * HG:(hg + 1) * HG, :] = out
    return full


# revision 10
# speedup vs baseline: 1.4195x; 1.0080x over previous
"""CrossHeadAttention Trainium2 kernel (8-core SPMD, data+head parallel).

Reference computation (per batch b):
    k = x_enc @ Wk ; v = x_enc @ Wv ; q = x @ Wq        (bias-free linears)
    wei = softmax((q @ k^T) / sqrt(1024))  per head
    out = wei @ v                                        -> [B, T, H, D]

Sharding: 8 cores = 2 batches x 4 head-groups (4 heads each). Each core
receives x[b], x_enc[b] (host-cast to bf16) and the 256-column slice of
Wq/Wk/Wv for its heads, and produces the unnormalized attention numerator
plus the softmax denominator; the host divides and gathers.

Per-core dataflow (all matmuls bf16 = 1 col/cycle on the PE):
  x, x_enc --HW DMA-transpose (bf16, 16x128 xbar tiles)--> xT/xeT in SBUF
    (zero PE cost; the PE never transposes activations)
  qT/kT[d,t] = W-stationary matmuls; psum->sbuf bf16 rounding on DVE
  vT chunks --DMA-transpose--> v_sb[s, st, head, d] (+ones column at d=64
    so the PV matmul also produces the softmax denominator)
  S^T[s,t] = kT^T qT per head (K=64, two heads row-packed in partitions)
  p = exp(S/32): split 10/16 exact on ScalarE (act table, psum->sbuf bf16)
    and 6/16 on DVE via a calibrated Schraudolph bit-trick: bf16 bits of
    exp2(x) ~ int16(round(128*log2(e)*x/32 + 16250)); the constant-factor
    part of its bias cancels in the softmax ratio, leaving ~1% noise on
    3/8 of the weights (measured end-to-end err ~7e-3 vs the 2e-2 gate)
  num^T[t, d+1] = p-stationary PV matmuls: out[t-block, 65] accumulated
    over s-tiles into a single psum bank (DVE pre-memset + start=False
    so four 65-wide accumulators share one bank without zero-region
    clobber); col 64 = denominator. DVE copies psum->sbuf, SWDGE stores.

Engine budget per core (TimelineSim): PE ~296k cycles (123us) = proj 98k
+ scores 131k + PV 67k; ACT ~85us exp; DVE ~85us (schrau + copies); DMA
~45us. The kernel is PE-bound; exp hides under the matmul stream.
"""

from contextlib import ExitStack

import numpy as np
import ml_dtypes

import concourse.bacc as bacc
import concourse.tile as tile
from concourse import mybir
from concourse.bass_utils import run_bass_kernel_spmd

# Problem constants (hardcoded per spec)
B = 2
T = 2048          # query length
S = 2048          # key/value length
C = 1024          # n_embd
H = 16            # total heads
D = 64            # head size
N_CORES = 8
HG = H // (N_CORES // B)       # heads per core = 4
DCORE = HG * D                 # 256 projected dims per core
P = 128                        # partitions
CT = C // P                    # 8 contraction tiles
NPAIR = HG // 2                # 2 head pairs per core
TCH = 512                      # t-chunk width in attention
NTCH = T // TCH                # 4
ST = S // P                    # 16 s-tiles
NCH = 4                        # 512-row input chunks

F32 = mybir.dt.float32
BF16 = mybir.dt.bfloat16
I16 = mybir.dt.int16
AF = mybir.ActivationFunctionType

SCALE = float(C) ** -0.5       # 1/32, folded into the exp activation

# Schraudolph exp for the DVE share: bf16 bitpattern of exp(s*SCALE) ~
# round(A*s + B); B = 16256 (bf16 exponent bias<<7) + m, m=-6 calibrated
# end-to-end for round-to-nearest int16 conversion.
SCH_A = 128.0 * float(np.log2(np.e)) * SCALE
SCH_B = 16256.0 - 6.0
N_ACT_SP = 5                   # s-tile-pairs 0..4 on ScalarE (exact exp)


def _build_body(nc, tc, x, xe, wq, wk, wv, o):
    with ExitStack() as ctx:
        big = ctx.enter_context(tc.tile_pool(name="big", bufs=1))

        # per-chunk contiguous tiles: the DMA xbar transpose requires a
        # contiguous output access pattern
        xT = [big.tile([P, CT, TCH], BF16, tag=f"xT{c}", name=f"xT{c}")
              for c in range(NCH)]
        xeT = [big.tile([P, CT, TCH], BF16, tag=f"xeT{c}", name=f"xeT{c}")
               for c in range(NCH)]
        kT = big.tile([P, NPAIR, S], BF16, tag="kT")
        qT = big.tile([P, NPAIR, T], BF16, tag="qT")
        # v, with a ones column appended per head (col D) for softmax sums
        v_sb = big.tile([P, ST, HG, D + 1], BF16, tag="v_sb")
        nc.vector.memset(v_sb[:, :, :, D], 1.0)

        # prime the ScalarE exp table at t=0 so the table load is off the
        # critical path of the first real exp
        dummy = big.tile([1, 2], F32, tag="dummy")
        nc.vector.memset(dummy, 0.0)
        nc.scalar.activation(out=dummy, in_=dummy, func=AF.Exp)

        # weights, host-staged as [P, CT, DCORE] bf16: direct DMA
        w_sbs = {}
        for name, wdram in (("wq", wq), ("wk", wk), ("wv", wv)):
            wsb = big.tile([P, CT, DCORE], BF16, tag=f"{name}_sb")
            nc.gpsimd.dma_start(out=wsb, in_=wdram)
            w_sbs[name] = wsb

        # activation transposes DRAM->SBUF on the DMA xbar, chunked for
        # pipelining with the projections
        for c in range(NCH):
            nc.sync.dma_start_transpose(xT[c], x[c * TCH:(c + 1) * TCH, :])
            nc.sync.dma_start_transpose(xeT[c], xe[c * TCH:(c + 1) * TCH, :])

        def proj(ps, wname, pair, src, c):
            w = w_sbs[wname]
            for ct in range(CT):
                nc.tensor.matmul(
                    ps, w[:, ct, pair * P:(pair + 1) * P], src[c][:, ct, :],
                    start=(ct == 0), stop=(ct == CT - 1))

        with tc.tile_pool(name="pps", bufs=3, space="PSUM") as pps, \
             tc.tile_pool(name="vtc", bufs=2) as vtc:
            copy_engines = [nc.scalar, nc.vector]
            for c in range(NCH):
                csl = slice(c * TCH, (c + 1) * TCH)
                for pair in range(NPAIR):
                    ps = pps.tile([P, TCH], F32, tag="pps", name="qps")
                    proj(ps, "wq", pair, xT, c)
                    copy_q = copy_engines[pair].copy if pair == 0 else \
                        (lambda out, in_: nc.vector.tensor_copy(out=out,
                                                                in_=in_))
                    copy_q(out=qT[:, pair, csl], in_=ps)
                for pair in range(NPAIR):
                    ps = pps.tile([P, TCH], F32, tag="pps", name="kps")
                    proj(ps, "wk", pair, xeT, c)
                    if pair == 0:
                        nc.scalar.copy(out=kT[:, pair, csl], in_=ps)
                    else:
                        nc.vector.tensor_copy(out=kT[:, pair, csl], in_=ps)
                for pair in range(NPAIR):
                    ps = pps.tile([P, TCH], F32, tag="pps", name="vps")
                    proj(ps, "wv", pair, xeT, c)
                    vt = vtc.tile([P, TCH], BF16, tag="vt", name="vt")
                    nc.vector.tensor_copy(out=vt, in_=ps)
                    for h2 in range(2):
                        # [d, s-chunk] -> contiguous [s%128, st, d], then a
                        # DVE copy into v_sb's 65-strided head slot
                        vtr = vtc.tile([P, 4, D], BF16, tag="vtr",
                                       name="vtr")
                        nc.sync.dma_start_transpose(
                            vtr, vt[h2 * D:(h2 + 1) * D, :])
                        nc.vector.tensor_copy(
                            out=v_sb[:, c * 4:(c + 1) * 4, 2 * pair + h2,
                                     0:D],
                            in_=vtr)

        # Attention: flat software-pipelined loop over (group, s-tile-pair)
        # units. Scores are emitted LOOK=2 units ahead of the exp/PV that
        # consume them, so the PE never sits behind the ~1us exp latency;
        # the PE stream is ... sc(i+2) pv(i) sc(i+3) pv(i+1) ...
        LOOK = 2
        NSP = ST // 2
        units = [(tch, pair, h2, sp)
                 for tch in range(NTCH)
                 for pair in range(NPAIR)
                 for h2 in range(2)
                 for sp in range(NSP)]

        with tc.tile_pool(name="sps", bufs=LOOK + 1, space="PSUM") as sps, \
             tc.tile_pool(name="pvps", bufs=2, space="PSUM") as pvps, \
             tc.tile_pool(name="psb", bufs=LOOK + 2) as psb, \
             tc.tile_pool(name="osb", bufs=3) as osb:
            s_tiles = {}
            pv_tiles = {}
            NG = len(units) // NSP

            def emit_scores(u):
                tch, pair, h2, sp = u
                tsl = slice(tch * TCH, (tch + 1) * TCH)
                s_ps = sps.tile([P, 2, TCH], F32, tag="s", name="s_ps")
                for j in range(2):
                    st = 2 * sp + j
                    nc.tensor.matmul(
                        s_ps[:, j, :],
                        kT[h2 * D:(h2 + 1) * D, pair, st * P:(st + 1) * P],
                        qT[h2 * D:(h2 + 1) * D, pair, tsl],
                        start=True, stop=True,
                        tile_position=(h2 * D, 0))
                s_tiles[u] = s_ps

            def emit_memset(g):
                # four 65-wide accumulators packed in one psum bank: DVE
                # memset + start=False chains avoid the per-chain 2KB
                # zero-region clobbering bank neighbours
                pv_tiles[g] = pvps.tile([P, 4, D + 1], F32, tag="pv",
                                        name="pv")
                nc.vector.memset(pv_tiles[g], 0.0)

            def emit_drain(g):
                gtch, gpair, gh2, _ = units[g * NSP]
                o_t = osb.tile([P, 4, D + 1], F32, tag="o", name="o_t")
                nc.vector.tensor_copy(out=o_t, in_=pv_tiles.pop(g))
                # SWDGE keeps stores off the SP queue feeding loads
                nc.gpsimd.dma_start(out=o[gtch, 2 * gpair + gh2], in_=o_t)

            emit_memset(0)
            for u in units[:LOOK]:
                emit_scores(u)
            for i, u in enumerate(units):
                tch, pair, h2, sp = u
                h = 2 * pair + h2
                g = i // NSP
                if i + LOOK < len(units):
                    emit_scores(units[i + LOOK])
                s_ps = s_tiles.pop(u)
                p_t = psb.tile([P, 2, TCH], BF16, tag="p", name="p_t")
                if sp >= NSP - N_ACT_SP:
                    # late s-tile-pairs on ScalarE: its 5-exp chain drains
                    # while the DVE (which owns the next group's early
                    # pairs) is already free at the group boundary
                    nc.scalar.activation(out=p_t, in_=s_ps, func=AF.Exp,
                                         scale=SCALE)
                else:
                    nc.vector.tensor_scalar(
                        out=p_t.bitcast(I16), in0=s_ps,
                        scalar1=SCH_A, scalar2=SCH_B,
                        op0=mybir.AluOpType.mult,
                        op1=mybir.AluOpType.add)
                pv = pv_tiles[g]
                for j in range(2):
                    st = 2 * sp + j
                    for tb in range(4):
                        nc.tensor.matmul(
                            pv[:, tb, :],
                            p_t[:, j, tb * P:(tb + 1) * P],
                            v_sb[:, st, h, :],
                            start=False, stop=(st == ST - 1),
                            skip_group_check=True)
                if sp == 3:
                    # DVE idle window (ScalarE owns sp>=3): retire the
                    # previous group and zero the next group's bank
                    if g >= 1:
                        emit_drain(g - 1)
                    if g + 1 < NG:
                        emit_memset(g + 1)
            emit_drain(NG - 1)


def build_program():
    nc = bacc.Bacc("TRN2", target_bir_lowering=False, debug=False,
                   num_devices=N_CORES)

    x = nc.dram_tensor("x", [T, C], BF16, kind="ExternalInput").ap()
    xe = nc.dram_tensor("xe", [S, C], BF16, kind="ExternalInput").ap()
    wq = nc.dram_tensor("wq", [P, CT, DCORE], BF16, kind="ExternalInput").ap()
    wk = nc.dram_tensor("wk", [P, CT, DCORE], BF16, kind="ExternalInput").ap()
    wv = nc.dram_tensor("wv", [P, CT, DCORE], BF16, kind="ExternalInput").ap()
    # per (tch, h): [t%512 partition, t-block, d+1]; col D = softmax denom
    o = nc.dram_tensor("o", [NTCH, HG, P, 4, D + 1], F32,
                       kind="ExternalOutput").ap()

    with tile.TileContext(nc) as tc:
        _build_body(nc, tc, x, xe, wq, wk, wv, o)
    nc.compile()
    return nc


_NC_CACHE = None


def _get_program():
    global _NC_CACHE
    if _NC_CACHE is None:
        _NC_CACHE = build_program()
    return _NC_CACHE


def _stage_w(wfull, csl):
    # [1024, 256] slice -> [P, CT, DCORE] bf16 with w[p, ct, d] = W[ct*128+p, d]
    wslc = np.ascontiguousarray(wfull[:, csl]).astype(ml_dtypes.bfloat16)
    return np.ascontiguousarray(wslc.reshape(CT, P, DCORE).transpose(1, 0, 2))


def kernel(x_enc, x, Wk, Wq, Wv):
    x_enc = np.asarray(x_enc, dtype=np.float32)
    x = np.asarray(x, dtype=np.float32)
    Wk = np.asarray(Wk, dtype=np.float32)
    Wq = np.asarray(Wq, dtype=np.float32)
    Wv = np.asarray(Wv, dtype=np.float32)

    nc = _get_program()
    in_maps = []
    for core in range(N_CORES):
        b, hg = divmod(core, N_CORES // B)
        csl = slice(hg * DCORE, (hg + 1) * DCORE)
        in_maps.append({
            "x": np.ascontiguousarray(x[b]).astype(ml_dtypes.bfloat16),
            "xe": np.ascontiguousarray(x_enc[b]).astype(ml_dtypes.bfloat16),
            "wq": _stage_w(Wq, csl),
            "wk": _stage_w(Wk, csl),
            "wv": _stage_w(Wv, csl),
        })
    res = run_bass_kernel_spmd(nc, in_maps, list(range(N_CORES)))

    full = np.empty((B, T, H, D), dtype=np.float32)
    for core in range(N_CORES):
        b, hg = divmod(core, N_CORES // B)
        o = res.results[core]["o"]          # [NTCH, HG, P, 4, D+1] f32
        num = o[..., :D]
        den = o[..., D]
        out = num / den[..., None]          # [tch, h, p, tb, d]
        out = out.transpose(0, 3, 2, 1, 4).reshape(T, HG, D)
        full[b, :, hg * HG:(hg + 1) * HG, :] = out
    return full


# revision 11
# speedup vs baseline: 1.4726x; 1.0374x over previous
"""CrossHeadAttention Trainium2 kernel (8-core SPMD, data+head parallel).

Reference computation (per batch b):
    k = x_enc @ Wk ; v = x_enc @ Wv ; q = x @ Wq        (bias-free linears)
    wei = softmax((q @ k^T) / sqrt(1024))  per head
    out = wei @ v                                        -> [B, T, H, D]

Sharding: 8 cores = 2 batches x 4 head-groups (4 heads each). Each core
receives x[b], x_enc[b] (host-cast to bf16) and the 256-column slice of
Wq/Wk/Wv for its heads, and produces the unnormalized attention numerator
plus the softmax denominator; the host divides and gathers.

Per-core dataflow (all matmuls bf16 = 1 col/cycle on the PE):
  x, x_enc --HW DMA-transpose (bf16, 16x128 xbar tiles)--> xT/xeT in SBUF
    (zero PE cost; the PE never transposes activations)
  qT/kT[d,t] = W-stationary matmuls; psum->sbuf bf16 rounding on DVE
  vT chunks --DMA-transpose--> v_sb[s, st, head, d] (+ones column at d=64
    so the PV matmul also produces the softmax denominator)
  S^T[s,t] = kT^T qT per head (K=64, two heads row-packed in partitions)
  p = exp(S/32): split 10/16 exact on ScalarE (act table, psum->sbuf bf16)
    and 6/16 on DVE via a calibrated Schraudolph bit-trick: bf16 bits of
    exp2(x) ~ int16(round(128*log2(e)*x/32 + 16250)); the constant-factor
    part of its bias cancels in the softmax ratio, leaving ~1% noise on
    3/8 of the weights (measured end-to-end err ~7e-3 vs the 2e-2 gate)
  num^T[t, d+1] = p-stationary PV matmuls: out[t-block, 65] accumulated
    over s-tiles into a single psum bank (DVE pre-memset + start=False
    so four 65-wide accumulators share one bank without zero-region
    clobber); col 64 = denominator. DVE copies psum->sbuf, SWDGE stores.

Engine budget per core (TimelineSim): PE ~296k cycles (123us) = proj 98k
+ scores 131k + PV 67k; ACT ~85us exp; DVE ~85us (schrau + copies); DMA
~45us. The kernel is PE-bound; exp hides under the matmul stream.
"""

from contextlib import ExitStack

import numpy as np
import ml_dtypes

import concourse.bacc as bacc
import concourse.tile as tile
from concourse import mybir
from concourse.bass_utils import run_bass_kernel_spmd

# Problem constants (hardcoded per spec)
B = 2
T = 2048          # query length
S = 2048          # key/value length
C = 1024          # n_embd
H = 16            # total heads
D = 64            # head size
N_CORES = 8
HG = H // (N_CORES // B)       # heads per core = 4
DCORE = HG * D                 # 256 projected dims per core
P = 128                        # partitions
CT = C // P                    # 8 contraction tiles
NPAIR = HG // 2                # 2 head pairs per core
TCH = 512                      # t-chunk width in attention
NTCH = T // TCH                # 4
ST = S // P                    # 16 s-tiles
NCH = 4                        # 512-row input chunks

F32 = mybir.dt.float32
BF16 = mybir.dt.bfloat16
I16 = mybir.dt.int16
AF = mybir.ActivationFunctionType

SCALE = float(C) ** -0.5       # 1/32, folded into the exp activation

# Schraudolph exp for the DVE share: bf16 bitpattern of exp(s*SCALE) ~
# round(A*s + B); B = 16256 (bf16 exponent bias<<7) + m, m=-6 calibrated
# end-to-end for round-to-nearest int16 conversion.
SCH_A = 128.0 * float(np.log2(np.e)) * SCALE
SCH_B = 16256.0 - 6.0
DVE_SP = (1, 3, 5)             # s-tile-pairs on DVE (Schraudolph exp);
                               # the rest run exact exp on ScalarE


def _build_body(nc, tc, x, xe, wq, wk, wv, o):
    with ExitStack() as ctx:
        big = ctx.enter_context(tc.tile_pool(name="big", bufs=1))

        # per-chunk contiguous tiles: the DMA xbar transpose requires a
        # contiguous output access pattern
        xT = [big.tile([P, CT, TCH], BF16, tag=f"xT{c}", name=f"xT{c}")
              for c in range(NCH)]
        xeT = [big.tile([P, CT, TCH], BF16, tag=f"xeT{c}", name=f"xeT{c}")
               for c in range(NCH)]
        kT = big.tile([P, NPAIR, S], BF16, tag="kT")
        qT = big.tile([P, NPAIR, T], BF16, tag="qT")
        # v, with a ones column appended per head (col D) for softmax sums
        v_sb = big.tile([P, ST, HG, D + 1], BF16, tag="v_sb")
        nc.vector.memset(v_sb[:, :, :, D], 1.0)

        # prime the ScalarE exp table at t=0 so the table load is off the
        # critical path of the first real exp
        dummy = big.tile([1, 2], F32, tag="dummy")
        nc.vector.memset(dummy, 0.0)
        nc.scalar.activation(out=dummy, in_=dummy, func=AF.Exp)

        # activation transposes DRAM->SBUF on the DMA xbar, chunked for
        # pipelining with the projections; W loads slot in right after the
        # first x chunk so qT0 starts early and kv0 is not delayed
        w_sbs = {}
        nc.sync.dma_start_transpose(xT[0], x[0:TCH, :])
        for name, wdram in (("wq", wq), ("wk", wk), ("wv", wv)):
            wsb = big.tile([P, CT, DCORE], BF16, tag=f"{name}_sb",
                           name=f"{name}_sb")
            nc.sync.dma_start(out=wsb, in_=wdram)
            w_sbs[name] = wsb
        nc.sync.dma_start_transpose(xeT[0], xe[0:TCH, :])
        for c in range(1, NCH):
            nc.sync.dma_start_transpose(xT[c], x[c * TCH:(c + 1) * TCH, :])
            nc.sync.dma_start_transpose(xeT[c], xe[c * TCH:(c + 1) * TCH, :])

        def proj(ps, wname, pair, src, c):
            w = w_sbs[wname]
            for ct in range(CT):
                nc.tensor.matmul(
                    ps, w[:, ct, pair * P:(pair + 1) * P], src[c][:, ct, :],
                    start=(ct == 0), stop=(ct == CT - 1))

        with tc.tile_pool(name="pps", bufs=3, space="PSUM") as pps, \
             tc.tile_pool(name="vtc", bufs=2) as vtc:
            copy_engines = [nc.scalar, nc.vector]
            for c in range(NCH):
                csl = slice(c * TCH, (c + 1) * TCH)
                for pair in range(NPAIR):
                    ps = pps.tile([P, TCH], F32, tag="pps", name="qps")
                    proj(ps, "wq", pair, xT, c)
                    copy_q = copy_engines[pair].copy if pair == 0 else \
                        (lambda out, in_: nc.vector.tensor_copy(out=out,
                                                                in_=in_))
                    copy_q(out=qT[:, pair, csl], in_=ps)
                for pair in range(NPAIR):
                    ps = pps.tile([P, TCH], F32, tag="pps", name="kps")
                    proj(ps, "wk", pair, xeT, c)
                    if pair == 0:
                        nc.scalar.copy(out=kT[:, pair, csl], in_=ps)
                    else:
                        nc.vector.tensor_copy(out=kT[:, pair, csl], in_=ps)
                for pair in range(NPAIR):
                    ps = pps.tile([P, TCH], F32, tag="pps", name="vps")
                    proj(ps, "wv", pair, xeT, c)
                    vt = vtc.tile([P, TCH], BF16, tag="vt", name="vt")
                    nc.vector.tensor_copy(out=vt, in_=ps)
                    for h2 in range(2):
                        # [d, s-chunk] -> contiguous [s%128, st, d], then a
                        # DVE copy into v_sb's 65-strided head slot
                        vtr = vtc.tile([P, 4, D], BF16, tag="vtr",
                                       name="vtr")
                        nc.sync.dma_start_transpose(
                            vtr, vt[h2 * D:(h2 + 1) * D, :])
                        nc.vector.tensor_copy(
                            out=v_sb[:, c * 4:(c + 1) * 4, 2 * pair + h2,
                                     0:D],
                            in_=vtr)

        # Attention: flat software-pipelined loop over (group, s-tile-pair)
        # units. Scores are emitted LOOK=2 units ahead of the exp/PV that
        # consume them, so the PE never sits behind the ~1us exp latency;
        # the PE stream is ... sc(i+2) pv(i) sc(i+3) pv(i+1) ...
        LOOK = 2
        NSP = ST // 2
        units = [(tch, pair, h2, sp)
                 for tch in range(NTCH)
                 for pair in range(NPAIR)
                 for h2 in range(2)
                 for sp in range(NSP)]

        with tc.tile_pool(name="sps", bufs=LOOK + 1, space="PSUM") as sps, \
             tc.tile_pool(name="pvps", bufs=2, space="PSUM") as pvps, \
             tc.tile_pool(name="psb", bufs=LOOK + 2) as psb, \
             tc.tile_pool(name="osb", bufs=3) as osb:
            s_tiles = {}
            pv_tiles = {}
            NG = len(units) // NSP

            def emit_scores(u):
                tch, pair, h2, sp = u
                tsl = slice(tch * TCH, (tch + 1) * TCH)
                s_ps = sps.tile([P, 2, TCH], F32, tag="s", name="s_ps")
                for j in range(2):
                    st = 2 * sp + j
                    nc.tensor.matmul(
                        s_ps[:, j, :],
                        kT[h2 * D:(h2 + 1) * D, pair, st * P:(st + 1) * P],
                        qT[h2 * D:(h2 + 1) * D, pair, tsl],
                        start=True, stop=True,
                        tile_position=(h2 * D, 0))
                s_tiles[u] = s_ps

            def emit_memset(g):
                # four 65-wide accumulators packed in one psum bank: DVE
                # memset + start=False chains avoid the per-chain 2KB
                # zero-region clobbering bank neighbours
                pv_tiles[g] = pvps.tile([P, 4, D + 1], F32, tag="pv",
                                        name="pv")
                nc.vector.memset(pv_tiles[g], 0.0)

            def emit_drain(g):
                gtch, gpair, gh2, _ = units[g * NSP]
                o_t = osb.tile([P, 4, D + 1], F32, tag="o", name="o_t")
                nc.vector.tensor_copy(out=o_t, in_=pv_tiles.pop(g))
                # SWDGE keeps stores off the SP queue feeding loads
                nc.gpsimd.dma_start(out=o[gtch, 2 * gpair + gh2], in_=o_t)

            emit_memset(0)
            for u in units[:LOOK]:
                emit_scores(u)
            for i, u in enumerate(units):
                tch, pair, h2, sp = u
                h = 2 * pair + h2
                g = i // NSP
                if i + LOOK < len(units):
                    emit_scores(units[i + LOOK])
                s_ps = s_tiles.pop(u)
                p_t = psb.tile([P, 2, TCH], BF16, tag="p", name="p_t")
                if sp not in DVE_SP:
                    # ScalarE/DVE alternate through the group so both exp
                    # chains start at the top of the group and neither
                    # engine sits idle waiting for late scores
                    nc.scalar.activation(out=p_t, in_=s_ps, func=AF.Exp,
                                         scale=SCALE)
                else:
                    nc.vector.tensor_scalar(
                        out=p_t.bitcast(I16), in0=s_ps,
                        scalar1=SCH_A, scalar2=SCH_B,
                        op0=mybir.AluOpType.mult,
                        op1=mybir.AluOpType.add)
                pv = pv_tiles[g]
                for j in range(2):
                    st = 2 * sp + j
                    for tb in range(4):
                        nc.tensor.matmul(
                            pv[:, tb, :],
                            p_t[:, j, tb * P:(tb + 1) * P],
                            v_sb[:, st, h, :],
                            start=False, stop=(st == ST - 1),
                            skip_group_check=True)
                if sp == 6:
                    # DVE slack window (its last exp was sp==5): retire the
                    # previous group and zero the next group's bank
                    if g >= 1:
                        emit_drain(g - 1)
                    if g + 1 < NG:
                        emit_memset(g + 1)
            emit_drain(NG - 1)


def build_program():
    nc = bacc.Bacc("TRN2", target_bir_lowering=False, debug=False,
                   num_devices=N_CORES)

    x = nc.dram_tensor("x", [T, C], BF16, kind="ExternalInput").ap()
    xe = nc.dram_tensor("xe", [S, C], BF16, kind="ExternalInput").ap()
    wq = nc.dram_tensor("wq", [P, CT, DCORE], BF16, kind="ExternalInput").ap()
    wk = nc.dram_tensor("wk", [P, CT, DCORE], BF16, kind="ExternalInput").ap()
    wv = nc.dram_tensor("wv", [P, CT, DCORE], BF16, kind="ExternalInput").ap()
    # per (tch, h): [t%512 partition, t-block, d+1]; col D = softmax denom
    o = nc.dram_tensor("o", [NTCH, HG, P, 4, D + 1], F32,
                       kind="ExternalOutput").ap()

    with tile.TileContext(nc) as tc:
        _build_body(nc, tc, x, xe, wq, wk, wv, o)
    nc.compile()
    return nc


_NC_CACHE = None


def _get_program():
    global _NC_CACHE
    if _NC_CACHE is None:
        _NC_CACHE = build_program()
    return _NC_CACHE


def _stage_w(wfull, csl):
    # [1024, 256] slice -> [P, CT, DCORE] bf16 with w[p, ct, d] = W[ct*128+p, d]
    wslc = np.ascontiguousarray(wfull[:, csl]).astype(ml_dtypes.bfloat16)
    return np.ascontiguousarray(wslc.reshape(CT, P, DCORE).transpose(1, 0, 2))


def kernel(x_enc, x, Wk, Wq, Wv):
    x_enc = np.asarray(x_enc, dtype=np.float32)
    x = np.asarray(x, dtype=np.float32)
    Wk = np.asarray(Wk, dtype=np.float32)
    Wq = np.asarray(Wq, dtype=np.float32)
    Wv = np.asarray(Wv, dtype=np.float32)

    nc = _get_program()
    in_maps = []
    for core in range(N_CORES):
        b, hg = divmod(core, N_CORES // B)
        csl = slice(hg * DCORE, (hg + 1) * DCORE)
        in_maps.append({
            "x": np.ascontiguousarray(x[b]).astype(ml_dtypes.bfloat16),
            "xe": np.ascontiguousarray(x_enc[b]).astype(ml_dtypes.bfloat16),
            "wq": _stage_w(Wq, csl),
            "wk": _stage_w(Wk, csl),
            "wv": _stage_w(Wv, csl),
        })
    res = run_bass_kernel_spmd(nc, in_maps, list(range(N_CORES)))

    full = np.empty((B, T, H, D), dtype=np.float32)
    for core in range(N_CORES):
        b, hg = divmod(core, N_CORES // B)
        o = res.results[core]["o"]          # [NTCH, HG, P, 4, D+1] f32
        num = o[..., :D]
        den = o[..., D]
        out = num / den[..., None]          # [tch, h, p, tb, d]
        out = out.transpose(0, 3, 2, 1, 4).reshape(T, HG, D)
        full[b, :, hg * HG:(hg + 1) * HG, :] = out
    return full


# revision 23
# speedup vs baseline: 1.7172x; 1.1661x over previous
"""CrossHeadAttention Trainium2 kernel (8-core SPMD, data+head parallel).

Reference computation (per batch b):
    k = x_enc @ Wk ; v = x_enc @ Wv ; q = x @ Wq        (bias-free linears)
    wei = softmax((q @ k^T) / sqrt(1024))  per head
    out = wei @ v                                        -> [B, T, H, D]

Sharding: 8 cores = 2 batches x 4 head-groups (4 heads each). Each core
receives x[b], x_enc[b] (host-cast to bf16) and the 256-column slice of
Wq/Wk/Wv for its heads, and produces the unnormalized attention numerator
plus the softmax denominator; the host divides and gathers.

Per-core dataflow (all matmuls bf16 = 1 col/cycle on the PE):
  x, x_enc --HW DMA-transpose (bf16, 16x128 xbar tiles)--> xT/xeT in SBUF
    (zero PE cost; the PE never transposes activations)
  qT/kT[d,t] = W-stationary matmuls; psum->sbuf bf16 rounding on DVE
  vT chunks --DMA-transpose--> v_sb[s, st, head, d] (+ones column at d=64
    so the PV matmul also produces the softmax denominator)
  S^T[s,t] = kT^T qT per head (K=64, two heads row-packed in partitions)
  p = exp(S/32): split 10/16 exact on ScalarE (act table, psum->sbuf bf16)
    and 6/16 on DVE via a calibrated Schraudolph bit-trick: bf16 bits of
    exp2(x) ~ int16(round(128*log2(e)*x/32 + 16250)); the constant-factor
    part of its bias cancels in the softmax ratio, leaving ~1% noise on
    3/8 of the weights (measured end-to-end err ~7e-3 vs the 2e-2 gate)
  num^T[t, d+1] = p-stationary PV matmuls: out[t-block, 65] accumulated
    over s-tiles into a single psum bank (DVE pre-memset + start=False
    so four 65-wide accumulators share one bank without zero-region
    clobber); col 64 = denominator. DVE copies psum->sbuf, SWDGE stores.

Engine budget per core (TimelineSim): PE ~296k cycles (123us) = proj 98k
+ scores 131k + PV 67k; ACT ~85us exp; DVE ~85us (schrau + copies); DMA
~45us. The kernel is PE-bound; exp hides under the matmul stream.
"""

from contextlib import ExitStack

import numpy as np
import ml_dtypes

import concourse.bacc as bacc
import concourse.tile as tile
from concourse import mybir
from concourse.bass_utils import run_bass_kernel_spmd

# Problem constants (hardcoded per spec)
B = 2
T = 2048          # query length
S = 2048          # key/value length
C = 1024          # n_embd
H = 16            # total heads
D = 64            # head size
N_CORES = 8
HG = H // (N_CORES // B)       # heads per core = 4
DCORE = HG * D                 # 256 projected dims per core
P = 128                        # partitions
CT = C // P                    # 8 contraction tiles
NPAIR = HG // 2                # 2 head pairs per core
TCH = 512                      # t-chunk width in attention
NTCH = T // TCH                # 4
ST = S // P                    # 16 s-tiles
NCH = 4                        # 512-row input chunks

F32 = mybir.dt.float32
BF16 = mybir.dt.bfloat16
I16 = mybir.dt.int16
AF = mybir.ActivationFunctionType

SCALE = float(C) ** -0.5       # 1/32, folded into the exp activation

# Schraudolph exp for the DVE share: bf16 bitpattern of exp(s*SCALE) ~
# round(A*s + B); B = 16256 (bf16 exponent bias<<7) + m, m=-6 calibrated
# end-to-end for round-to-nearest int16 conversion.
SCH_A = 128.0 * float(np.log2(np.e)) * SCALE
SCH_B = 16256.0 - 6.0
DVE_SP = (1, 3, 5)             # s-tile-pairs on DVE (Schraudolph exp);
                               # the rest run exact exp on ScalarE
LOOK = 2                       # score-emission lookahead (units)
DRAIN_ON_ACT = True            # pv drain copy engine (ScalarE vs DVE)
DVE_SPLIT = True               # DVE exps as two half-tile instructions
SC_SPLIT = True                # emit trailing score half after pv burst


def _build_body(nc, tc, x, xe, wq, wk, wv, o):
    with ExitStack() as ctx:
        big = ctx.enter_context(tc.tile_pool(name="big", bufs=1))

        # per-chunk contiguous tiles: the DMA xbar transpose requires a
        # contiguous output access pattern
        xT = [big.tile([P, CT, TCH], BF16, tag=f"xT{c}", name=f"xT{c}")
              for c in range(NCH)]
        xeT = [big.tile([P, CT, TCH], BF16, tag=f"xeT{c}", name=f"xeT{c}")
               for c in range(NCH)]
        kT = big.tile([P, NPAIR, S], BF16, tag="kT")
        qT = big.tile([P, NPAIR, T], BF16, tag="qT")
        # v, with a ones column appended per head (col D) for softmax sums
        v_sb = big.tile([P, ST, HG, D + 1], BF16, tag="v_sb")
        nc.vector.memset(v_sb[:, :, :, D], 1.0)

        # prime the ScalarE exp table at t=0 so the table load is off the
        # critical path of the first real exp
        dummy = big.tile([1, 2], F32, tag="dummy")
        nc.vector.memset(dummy, 0.0)
        nc.scalar.activation(out=dummy, in_=dummy, func=AF.Exp)

        # DMA order on the serial xbar/DMA-engine resource: first x chunk,
        # then Wq (unblocks qT0), first xe chunk, Wk/Wv (unblocks kv0),
        # then the remaining chunks interleaved
        w_sbs = {}

        def load_w(name, wdram):
            wsb = big.tile([P, CT, DCORE], BF16, tag=f"{name}_sb",
                           name=f"{name}_sb")
            nc.sync.dma_start(out=wsb, in_=wdram)
            w_sbs[name] = wsb

        nc.sync.dma_start_transpose(xT[0], x[0:TCH, :])
        load_w("wq", wq)
        nc.sync.dma_start_transpose(xeT[0], xe[0:TCH, :])
        load_w("wk", wk)
        load_w("wv", wv)
        for c in range(1, NCH):
            nc.sync.dma_start_transpose(xT[c], x[c * TCH:(c + 1) * TCH, :])
            nc.sync.dma_start_transpose(xeT[c], xe[c * TCH:(c + 1) * TCH, :])

        vtc = ctx.enter_context(tc.tile_pool(name="vtc", bufs=2))
        # projection psums share the score pool's rotation: psum has only
        # 8 banks and scores + pv accumulators need all of them
        sps = ctx.enter_context(tc.tile_pool(name="sps", bufs=6,
                                             space="PSUM"))
        pps = sps
        pvps = ctx.enter_context(tc.tile_pool(name="pvps", bufs=2,
                                              space="PSUM"))
        psb = ctx.enter_context(tc.tile_pool(name="psb", bufs=8))
        osb = ctx.enter_context(tc.tile_pool(name="osb", bufs=4))

        def proj(ps, wname, pair, src, c):
            w = w_sbs[wname]
            for ct in range(CT):
                nc.tensor.matmul(
                    ps, w[:, ct, pair * P:(pair + 1) * P], src[c][:, ct, :],
                    start=(ct == 0), stop=(ct == CT - 1))

        def emit_qt(c):
            csl = slice(c * TCH, (c + 1) * TCH)
            for pair in range(NPAIR):
                ps = pps.tile([P, TCH], F32, tag="s", name="qps")
                proj(ps, "wq", pair, xT, c)
                if pair == 0:
                    nc.scalar.copy(out=qT[:, pair, csl], in_=ps)
                else:
                    nc.vector.tensor_copy(out=qT[:, pair, csl], in_=ps)

        def emit_kv(c):
            csl = slice(c * TCH, (c + 1) * TCH)
            for pair in range(NPAIR):
                ps = pps.tile([P, TCH], F32, tag="s", name="kps")
                proj(ps, "wk", pair, xeT, c)
                if pair == 0:
                    nc.scalar.copy(out=kT[:, pair, csl], in_=ps)
                else:
                    nc.vector.tensor_copy(out=kT[:, pair, csl], in_=ps)
            for pair in range(NPAIR):
                ps = pps.tile([P, TCH], F32, tag="s", name="vps")
                proj(ps, "wv", pair, xeT, c)
                vt = vtc.tile([P, TCH], BF16, tag="vt", name="vt")
                nc.vector.tensor_copy(out=vt, in_=ps)
                for h2 in range(2):
                    # [d, s-chunk] -> contiguous [s%128, st, d], then a
                    # DVE copy into v_sb's 65-strided head slot
                    vtr = vtc.tile([P, 4, D], BF16, tag="vtr", name="vtr")
                    nc.sync.dma_start_transpose(
                        vtr, vt[h2 * D:(h2 + 1) * D, :])
                    nc.vector.tensor_copy(
                        out=v_sb[:, c * 4:(c + 1) * 4, 2 * pair + h2, 0:D],
                        in_=vtr)

        # Attention: software-pipelined loop over (group, s-tile-pair)
        # units, with GROUPS PROCESSED IN INTERLEAVED PAIRS: units of two
        # head-groups alternate, doubling the PE work between any score->
        # exp->pv dependency chain so the in-order PE never stalls on the
        # ~0.6us exp instructions. Scores are emitted LOOK entries ahead.
        # The kv/qT projections for chunks 1-3 are spliced between the
        # first pair's units: the PE chews on them while later xe chunks
        # stream in, and the exp engines warm up before the steady state.
        NSP = ST // 2
        groups = [(tch, pair, h2)
                  for tch in range(NTCH)
                  for pair in range(NPAIR)
                  for h2 in range(2)]
        NG = len(groups)
        entries = [(g, sp) for g in range(NG) for sp in range(NSP)]
        s_tiles = {}
        pv_tiles = {}
        # kv chunk c feeds s-tiles 4c..4c+3 = sp pairs 2c..2c+1 of the
        # first group pair; qT chunks feed later tch blocks
        hooks = {1: [lambda: emit_kv(1)],
                 3: [lambda: emit_kv(2), lambda: emit_qt(1)],
                 5: [lambda: emit_kv(3), lambda: emit_qt(2)],
                 7: [lambda: emit_qt(3)]}

        def emit_score_half(e, j):
            # each half-entry (one s-tile) gets its own single-bank psum
            # tile: six tiles rotate, so the score->exp->pv->reuse round
            # trip never throttles the PE
            g, sp = e
            tch, pair, h2 = groups[g]
            tsl = slice(tch * TCH, (tch + 1) * TCH)
            s_ps = sps.tile([P, TCH], F32, tag="s", name="s_ps")
            s_tiles[(e, j)] = s_ps
            st = 2 * sp + j
            nc.tensor.matmul(
                s_ps,
                kT[h2 * D:(h2 + 1) * D, pair, st * P:(st + 1) * P],
                qT[h2 * D:(h2 + 1) * D, pair, tsl],
                start=True, stop=True,
                tile_position=(h2 * D, 0))

        def emit_pv_alloc(g):
            # four 65-wide accumulators packed in one (bank-aligned) psum
            # bank; the group's very first matmul runs start=True, whose
            # 2KB zero-region pending-zero covers all four chains, so no
            # memset is needed and the other chains accumulate with
            # start=False
            pv_tiles[g] = pvps.tile([P, 4, D + 1], F32, tag="pv", name="pv")

        def emit_drain(g):
            gtch, gpair, gh2 = groups[g]
            o_t = osb.tile([P, 4, D + 1], F32, tag="o", name="o_t")
            nc.scalar.copy(out=o_t, in_=pv_tiles.pop(g))
            # SWDGE keeps stores off the SP queue feeding loads
            nc.gpsimd.dma_start(out=o[gtch, 2 * gpair + gh2], in_=o_t)

        emit_qt(0)
        emit_kv(0)
        emit_pv_alloc(0)
        for e in entries[:LOOK]:
            emit_score_half(e, 0)
            emit_score_half(e, 1)
        for i, e in enumerate(entries):
            g, sp = e
            tch, pair, h2 = groups[g]
            h = 2 * pair + h2
            if g < 2 and i in hooks:
                for fn in hooks[i]:
                    fn()
            if i + LOOK < len(entries):
                emit_score_half(entries[i + LOOK], 0)
                emit_score_half(entries[i + LOOK], 1)
            # each entry's exp is split by s-tile half: even s-tiles get
            # exact exp on ScalarE, odd s-tiles the calibrated Schraudolph
            # on DVE; the halves run concurrently on both engines
            pv = pv_tiles[g]
            for j in range(2):
                st = 2 * sp + j
                s_ps = s_tiles.pop((e, j))
                p_t = psb.tile([P, TCH], BF16, tag="p", name="p_t")
                if j == 0:
                    nc.scalar.activation(out=p_t, in_=s_ps, func=AF.Exp,
                                         scale=SCALE)
                else:
                    nc.vector.tensor_scalar(
                        out=p_t.bitcast(I16), in0=s_ps,
                        scalar1=SCH_A, scalar2=SCH_B,
                        op0=mybir.AluOpType.mult, op1=mybir.AluOpType.add)
                for tb in range(4):
                    nc.tensor.matmul(
                        pv[:, tb, :],
                        p_t[:, tb * P:(tb + 1) * P],
                        v_sb[:, st, h, :],
                        start=(sp == 0 and j == 0 and tb == 0),
                        stop=(st == ST - 1),
                        skip_group_check=True)
            if sp == 6:
                # slack window late in each group: retire the previous
                # group and allocate the next one's bank
                if g >= 1:
                    emit_drain(g - 1)
                if g + 1 < NG:
                    emit_pv_alloc(g + 1)
        emit_drain(NG - 1)


def build_program():
    nc = bacc.Bacc("TRN2", target_bir_lowering=False, debug=False,
                   num_devices=N_CORES)

    x = nc.dram_tensor("x", [T, C], BF16, kind="ExternalInput").ap()
    xe = nc.dram_tensor("xe", [S, C], BF16, kind="ExternalInput").ap()
    wq = nc.dram_tensor("wq", [P, CT, DCORE], BF16, kind="ExternalInput").ap()
    wk = nc.dram_tensor("wk", [P, CT, DCORE], BF16, kind="ExternalInput").ap()
    wv = nc.dram_tensor("wv", [P, CT, DCORE], BF16, kind="ExternalInput").ap()
    # per (tch, h): [t%512 partition, t-block, d+1]; col D = softmax denom
    o = nc.dram_tensor("o", [NTCH, HG, P, 4, D + 1], F32,
                       kind="ExternalOutput").ap()

    with tile.TileContext(nc) as tc:
        _build_body(nc, tc, x, xe, wq, wk, wv, o)
    nc.compile()
    return nc


_NC_CACHE = None


def _get_program():
    global _NC_CACHE
    if _NC_CACHE is None:
        _NC_CACHE = build_program()
    return _NC_CACHE


def _stage_w(wfull, csl):
    # [1024, 256] slice -> [P, CT, DCORE] bf16 with w[p, ct, d] = W[ct*128+p, d]
    wslc = np.ascontiguousarray(wfull[:, csl]).astype(ml_dtypes.bfloat16)
    return np.ascontiguousarray(wslc.reshape(CT, P, DCORE).transpose(1, 0, 2))


def kernel(x_enc, x, Wk, Wq, Wv):
    x_enc = np.asarray(x_enc, dtype=np.float32)
    x = np.asarray(x, dtype=np.float32)
    Wk = np.asarray(Wk, dtype=np.float32)
    Wq = np.asarray(Wq, dtype=np.float32)
    Wv = np.asarray(Wv, dtype=np.float32)

    nc = _get_program()
    in_maps = []
    for core in range(N_CORES):
        b, hg = divmod(core, N_CORES // B)
        csl = slice(hg * DCORE, (hg + 1) * DCORE)
        in_maps.append({
            "x": np.ascontiguousarray(x[b]).astype(ml_dtypes.bfloat16),
            "xe": np.ascontiguousarray(x_enc[b]).astype(ml_dtypes.bfloat16),
            "wq": _stage_w(Wq, csl),
            "wk": _stage_w(Wk, csl),
            "wv": _stage_w(Wv, csl),
        })
    res = run_bass_kernel_spmd(nc, in_maps, list(range(N_CORES)))

    full = np.empty((B, T, H, D), dtype=np.float32)
    for core in range(N_CORES):
        b, hg = divmod(core, N_CORES // B)
        o = res.results[core]["o"]          # [NTCH, HG, P, 4, D+1] f32
        num = o[..., :D]
        den = o[..., D]
        out = num / den[..., None]          # [tch, h, p, tb, d]
        out = out.transpose(0, 3, 2, 1, 4).reshape(T, HG, D)
        full[b, :, hg * HG:(hg + 1) * HG, :] = out
    return full
